# revision 1
# baseline (speedup 1.0000x reference)
"""Trainium2 Bass kernel for nn_GATrAutoRegressorLoss.

Strategy (data-parallel over the hit axis N, 8 cores):
  - The dominant cost is the assignment BCE over (T=32, N=500000) logits.
    Each core gets H = N/8 = 62500 hits, packed as a (128, 15625) layout:
    partition p = j*32 + t, column f, hit = j*15625 + f.
  - The validity mask is folded into the logits on the PE: host-built fp8
    one-hot columns E (encoding c(hit) = #valid steps) hit a constant
    block-triangular L with value -96, accumulating -96*(t >= c) into PSUM;
    x rides in via a bf16 identity matmul (bf16 logits keep the final
    losses within ~5e-5 relative).  psA = x - 96*notM.
  - softplus = ln(1 + exp(.)) as two ACT passes (no native softplus table
    in this compiler): exp(psA) underflows to exactly 0 for masked elements
    so ln(1+u) contributes 0 there; accum_out gives free row-sums.  Exp and
    Ln are pinned to the one ACT function table that contains both
    (see _Bacc) so the Scalar engine loads its table exactly once.
  - The BCE "- x*z" term needs no extra pass structure: selected elements
    are always valid, so psA = x there, and one scalar_tensor_tensor
    psA * D (D the fp8 one-hot selector, read from SBUF) with accum_out
    yields sum_sel x exactly.
  - The small (T,B) losses (dir/mag/pid/charge/stop) are computed on-device
    from host-scattered dense planes, batched over contiguous plane groups;
    index bookkeeping (bincount, cumcount, scatter, argmax one-hots,
    denominators) is host-side numpy.
  - Per-core partial sums are returned and combined on the host in float64.
"""

import numpy as np

import concourse.bacc as bacc
import concourse.mybir as mybir
from concourse.tile import TileContext
from concourse.bass_utils import run_bass_kernel_spmd

F32 = mybir.dt.float32
BF16 = mybir.dt.bfloat16
F8 = mybir.dt.float8e4
NP_F8 = mybir.dt.np(F8)
NP_BF16 = mybir.dt.np(BF16)

T, B, N, NPFO = 32, 256, 500000, 4096
L_DIR, L_MAG, L_PID, L_CHG, L_ASN, L_STP = 1.0, 1.0, 1.0, 0.5, 1.0, 0.5

N_CORES = 8
H = N // N_CORES          # hits per core
J = 4                     # partition packing factor (J*T = 128)
HQ = H // J               # packed columns per core
P = J * T                 # 128 partitions
FCH = 2048                # chunk width (columns)
MMW = 512                 # one PSUM bank (512 f32 cols) per matmul
PEN = 96.0                # mask penalty; exp(x-96) underflows to 0

_CHUNKS = []
_c0 = 0
for _w in (1024, 1024):  # priming chunks: fill the pipeline sooner
    _CHUNKS.append((_c0, _w))
    _c0 += _w
while _c0 < HQ:
    _CHUNKS.append((_c0, min(FCH, HQ - _c0)))
    _c0 += FCH
NCH = len(_CHUNKS)
assert NCH <= 16

# small-loss planes, each (T*B,) flattened to (128, 64)
_PLANES = [
    "pm0", "pm1", "pm2", "gm0", "gm1", "gm2", "pp", "gp", "pch", "gch",
    "stopx", "stopz", "valid",
    "pid0", "pid1", "pid2", "pid3", "pid4",
    "poh0", "poh1", "poh2", "poh3", "poh4",
]
NPL = len(_PLANES)
SW = 64  # small-plane free width (T*B = 8192 = 128*64)

_nc_cache = None
last_result = None


class _Bacc(bacc.Bacc):
    """Bacc whose ACT-table chooser binds Exp/Ln to the one json table that
    contains both (natural_log_exp_and_others), so the Scalar engine never
    reloads function tables between exp and ln passes.  Table ids passed to
    the rust pass keep their act_info.json positions; only the advertised
    contents are narrowed, so codegen still loads the real (correct) table."""

    def insert_act_table_loads(self):
        from concourse.hw_specs import get_activation_tables

        has_activation = any(
            isinstance(i, mybir.InstActivation)
            for b in self.main_func.blocks
            for i in b.instructions
        )
        if not has_activation:
            return
        AF = mybir.ActivationFunctionType
        tables = []
        for name, fns in get_activation_tables(self.m.arch).items():
            if name != "natural_log_exp_and_others":
                fns = set(fns) - {AF.Exp, AF.Ln}
            tables.append((name, set(fns)))
        import bass_rust as _bass_rust

        _bass_rust.insert_act_table_loads(self, tables)


def _gen():
    nc = _Bacc(None, target_bir_lowering=False, debug=True)
    xh = nc.dram_tensor("xh", [P, HQ], BF16, kind="ExternalInput")
    ed8 = nc.dram_tensor("ed8", [P, 2 * HQ], F8, kind="ExternalInput")
    l8 = nc.dram_tensor("l8", [P, P], F8, kind="ExternalInput")
    ibf = nc.dram_tensor("ibf", [P, P], BF16, kind="ExternalInput")
    sm = nc.dram_tensor("sm", [P, NPL * SW], F32, kind="ExternalInput")
    partials = nc.dram_tensor("partials", [P, 40], F32, kind="ExternalOutput")

    AF = mybir.ActivationFunctionType
    OP = mybir.AluOpType

    with TileContext(nc) as tc:
        with (
            tc.tile_pool(name="cst", bufs=1) as cst,
            tc.tile_pool(name="io", bufs=4) as io,
            tc.tile_pool(name="wk", bufs=3) as wk,
            tc.tile_pool(name="ps", bufs=2, space="PSUM") as ps,
            tc.tile_pool(name="sml", bufs=1) as sml,
        ):
            lt = cst.tile([P, P], F8)
            ft = cst.tile([P, P], BF16)
            accA = cst.tile([P, 16], F32)
            accB = cst.tile([P, 16], F32)
            accS = cst.tile([P, 8], F32)
            nc.vector.memset(accA[:], 0.0)
            nc.vector.memset(accB[:], 0.0)
            nc.vector.memset(accS[:], 0.0)

            # ---------------- main loop: assignment loss ----------------

            edv = ed8.rearrange("p (r q) -> p r q", r=2)
            for ci, (c0, w) in enumerate(_CHUNKS):
                last = ci == len(_CHUNKS) - 1
                if ci % 2 == 0:
                    # one DMA pair covers two chunks
                    pw = w + (0 if last else _CHUNKS[ci + 1][1])
                    xht = io.tile([P, 2 * FCH], BF16, tag="xht")
                    edt = io.tile([P, 2, 2 * FCH], F8, tag="edt")
                    nc.sync.dma_start(out=xht[:, :pw], in_=xh[:, c0 : c0 + pw])
                    nc.sync.dma_start(
                        out=edt[:, :, :pw], in_=edv[:, :, c0 : c0 + pw]
                    )
                    poff = 0
                    ut = wk.tile([P, 2 * FCH], BF16, tag="ut")
                    uoff = 0
                if ci == 0:
                    nc.sync.dma_start(out=lt[:], in_=l8[:])
                    nc.sync.dma_start(out=ft[:], in_=ibf[:])

                psA = ps.tile([P, FCH], F32, tag="psA")
                h0 = 0
                while h0 < w:
                    hw = min(MMW, w - h0)
                    sl = slice(h0, h0 + hw)
                    sl2 = slice(poff + h0, poff + h0 + hw)
                    nc.tensor.matmul(
                        psA[:, sl], lt[:], edt[:, 0, sl2], start=True,
                        stop=False,
                    )
                    nc.tensor.matmul(
                        psA[:, sl], ft[:], xht[:, sl2], start=False, stop=True
                    )
                    h0 += hw

                nc.scalar.activation(
                    out=ut[:, uoff : uoff + w], in_=psA[:, :w], func=AF.Exp
                )
                uoff += w
                poff += w
                if ci % 2 == 1 or last:
                    st = wk.tile([P, 2 * FCH], BF16, tag="st")
                    nc.scalar.activation(
                        out=st[:, :uoff],
                        in_=ut[:, :uoff],
                        func=AF.Ln,
                        bias=1.0,
                        accum_out=accA[:, ci // 2 : ci // 2 + 1],
                    )
                    rt = wk.tile([P, 2 * FCH], BF16, tag="rt")
                    nc.vector.scalar_tensor_tensor(
                        out=rt[:, :poff],
                        in0=xht[:, :poff],
                        scalar=1.0,
                        in1=edt[:, 1, :poff],
                        op0=OP.mult,
                        op1=OP.mult,
                        accum_out=accB[:, ci // 2 : ci // 2 + 1],
                    )

                if ci == 3:
                    # ---- small (T,B) losses, batched over contiguous planes
                    smt = sml.tile([P, NPL * SW], F32)
                    nc.sync.dma_start(out=smt[:], in_=sm[:])
                    PLI = {n: i for i, n in enumerate(_PLANES)}

                    def reg(name, k=1):
                        i = PLI[name]
                        return smt[:, i * SW : (i + k) * SW]

                    def red(ap, k, op=OP.add):
                        # reduce over the k plane-groups of a (P, k*SW) region
                        o = sml.tile([P, SW], F32, name=f"red{_tmp_n[0]}",
                                     tag=f"red{_tmp_n[0]}")
                        _tmp_n[0] += 1
                        nc.vector.tensor_reduce(
                            out=o[:],
                            in_=ap.rearrange("p (k j) -> p j k", k=k),
                            axis=mybir.AxisListType.X,
                            op=op,
                        )
                        return o

                    _tmp_n = [0]

                    def tmp(w=SW):
                        _tmp_n[0] += 1
                        nm = f"tmp{_tmp_n[0]}"
                        return sml.tile([P, w], F32, name=nm, tag=nm)

                    valid = reg("valid")

                    # --- direction loss
                    sqv = tmp(6 * SW)
                    nc.scalar.activation(
                        out=sqv[:], in_=reg("pm0", 6), func=AF.Square
                    )
                    ssb = tmp(2 * SW)
                    nc.vector.tensor_reduce(
                        out=ssb[:, 0:SW],
                        in_=sqv[:, 0 : 3 * SW].rearrange("p (k j) -> p j k", k=3),
                        axis=mybir.AxisListType.X, op=OP.add,
                    )
                    nc.vector.tensor_reduce(
                        out=ssb[:, SW : 2 * SW],
                        in_=sqv[:, 3 * SW : 6 * SW].rearrange(
                            "p (k j) -> p j k", k=3
                        ),
                        axis=mybir.AxisListType.X, op=OP.add,
                    )
                    lnb = tmp(2 * SW)
                    nc.scalar.activation(out=lnb[:], in_=ssb[:], func=AF.Ln)
                    srb = tmp(2 * SW)
                    nc.scalar.activation(
                        out=srb[:], in_=lnb[:], func=AF.Exp, scale=0.5
                    )
                    nc.vector.tensor_scalar(
                        out=srb[:], in0=srb[:], scalar1=1e-8, scalar2=None,
                        op0=OP.max,
                    )
                    nc.vector.reciprocal(out=srb[:], in_=srb[:])
                    dmul = tmp(3 * SW)
                    nc.vector.tensor_mul(dmul[:], reg("pm0", 3), reg("gm0", 3))
                    dot = red(dmul[:], 3)
                    nc.vector.tensor_mul(dot[:], dot[:], srb[:, 0:SW])
                    nc.vector.tensor_mul(dot[:], dot[:], srb[:, SW : 2 * SW])
                    cv = tmp()
                    nc.vector.tensor_mul(cv[:], dot[:], valid)
                    o1 = tmp()
                    nc.vector.scalar_tensor_tensor(
                        out=o1[:], in0=cv[:], scalar=-1.0, in1=valid,
                        op0=OP.mult, op1=OP.add, accum_out=accS[:, 0:1],
                    )

                    # --- magnitude / charge (masked squared diffs)
                    dif = tmp(2 * SW)
                    nc.vector.tensor_sub(dif[:, 0:SW], reg("pp"), reg("gp"))
                    nc.vector.tensor_sub(
                        dif[:, SW : 2 * SW], reg("pch"), reg("gch")
                    )
                    dsq = tmp(2 * SW)
                    nc.scalar.activation(out=dsq[:], in_=dif[:], func=AF.Square)
                    for col, sl in ((1, slice(0, SW)), (2, slice(SW, 2 * SW))):
                        o = tmp()
                        nc.vector.scalar_tensor_tensor(
                            out=o[:], in0=dsq[:, sl], scalar=1.0, in1=valid,
                            op0=OP.mult, op1=OP.mult,
                            accum_out=accS[:, col : col + 1],
                        )

                    # --- pid cross entropy (direct logsumexp; |logits| small)
                    pexp = tmp(5 * SW)
                    nc.scalar.activation(
                        out=pexp[:], in_=reg("pid0", 5), func=AF.Exp
                    )
                    se = red(pexp[:], 5)
                    lse = tmp()
                    nc.scalar.activation(out=lse[:], in_=se[:], func=AF.Ln)
                    xm = tmp(5 * SW)
                    nc.vector.tensor_mul(xm[:], reg("pid0", 5), reg("poh0", 5))
                    xcls = red(xm[:], 5)
                    u1 = tmp()
                    nc.vector.scalar_tensor_tensor(
                        out=u1[:], in0=xcls[:], scalar=-1.0, in1=lse[:],
                        op0=OP.mult, op1=OP.add,
                    )
                    o2 = tmp()
                    nc.vector.scalar_tensor_tensor(
                        out=o2[:], in0=u1[:], scalar=1.0, in1=valid,
                        op0=OP.mult, op1=OP.mult, accum_out=accS[:, 3:4],
                    )

                    # --- stop BCE over all (T,B)
                    usp = tmp()
                    nc.scalar.activation(out=usp[:], in_=reg("stopx"),
                                         func=AF.Exp)
                    spv = tmp()
                    nc.scalar.activation(out=spv[:], in_=usp[:], func=AF.Ln,
                                         bias=1.0)
                    xz = tmp()
                    nc.vector.tensor_mul(xz[:], reg("stopx"), reg("stopz"))
                    o3 = tmp()
                    nc.vector.scalar_tensor_tensor(
                        out=o3[:], in0=xz[:], scalar=-1.0, in1=spv[:],
                        op0=OP.mult, op1=OP.add, accum_out=accS[:, 4:5],
                    )
                elif ci == 8:
                    nc.sync.dma_start(
                        out=partials[:, 0:8], in_=accA[:, 0:8]
                    )
                    nc.sync.dma_start(
                        out=partials[:, 16:24], in_=accB[:, 0:8]
                    )

            nc.sync.dma_start(out=partials[:, 8:16], in_=accA[:, 8:16])
            nc.sync.dma_start(out=partials[:, 24:32], in_=accB[:, 8:16])
            nc.sync.dma_start(out=partials[:, 32:40], in_=accS[:])
    nc.finalize()
    return nc


def _get_nc():
    global _nc_cache
    if _nc_cache is None:
        _nc_cache = _gen()
    return _nc_cache


def _cumcount(gb):
    n = gb.shape[0]
    order = np.argsort(gb, kind="stable")
    sb = gb[order]
    first = np.searchsorted(sb, sb, side="left")
    cum = np.arange(n) - first
    out = np.zeros(n, dtype=np.int64)
    out[order] = cum
    return out


def kernel(**inputs):
    pfo_momentum = np.asarray(inputs["pfo_momentum"], np.float32)
    pfo_p_mod = np.asarray(inputs["pfo_p_mod"], np.float32)
    pfo_pid = np.asarray(inputs["pfo_pid"], np.float32)
    pfo_charge = np.asarray(inputs["pfo_charge"], np.float32)
    al = np.asarray(inputs["assignments_logits"], np.float32).reshape(T, N)
    stop_logits = np.asarray(inputs["stop_logits"], np.float32)
    gt_momentum = np.asarray(inputs["gt_momentum"], np.float32)
    gt_p_mod = np.asarray(inputs["gt_p_mod"], np.float32)
    gt_pid = np.asarray(inputs["gt_pid"], np.float32)
    gt_charge = np.asarray(inputs["gt_charge"], np.float32)
    gt_batch = np.asarray(inputs["gt_batch"]).astype(np.int64)
    hit_to_pfo = np.asarray(inputs["hit_to_pfo"]).astype(np.int64)
    hit_batch = np.asarray(inputs["hit_batch"]).astype(np.int64)

    # ---- host index bookkeeping ----
    ppe = np.bincount(gt_batch, minlength=B)[:B]                  # (B,)
    cmin = np.minimum(ppe[hit_batch], T)                          # (N,)
    w = hit_to_pfo < cmin                                         # (N,) bool
    assign_den = max(float(cmin.sum()), 1.0)

    step_idx = _cumcount(gt_batch)
    keep = step_idx < T
    si, gb = step_idx[keep], gt_batch[keep]

    def scat(vals):
        out = np.zeros((T, B) + vals.shape[1:], np.float32)
        out[si, gb] = vals[keep]
        return out

    gt_mom_tb = scat(gt_momentum)
    gt_pmod_tb = scat(gt_p_mod)
    gt_pid_tb = scat(gt_pid)
    gt_chg_tb = scat(gt_charge)

    steps = np.arange(T)[:, None]
    valid = (steps < ppe[None, :]).astype(np.float32)             # (T,B)
    vcnt = max(float(valid.sum()), 1.0)
    gt_stop = (steps >= ppe[None, :]).astype(np.float32)
    gt_cls = np.argmax(gt_pid_tb, axis=-1)                        # (T,B)
    poh = np.zeros((T, B, 5), np.float32)
    np.put_along_axis(poh, gt_cls[..., None], 1.0, axis=-1)

    # ---- per-core device inputs ----
    def pack_plane(a):
        return np.ascontiguousarray(a.reshape(P, SW))

    planes = {
        "pm0": pfo_momentum[..., 0], "pm1": pfo_momentum[..., 1],
        "pm2": pfo_momentum[..., 2],
        "gm0": gt_mom_tb[..., 0], "gm1": gt_mom_tb[..., 1],
        "gm2": gt_mom_tb[..., 2],
        "pp": pfo_p_mod[..., 0], "gp": gt_pmod_tb[..., 0],
        "pch": pfo_charge[..., 0], "gch": gt_chg_tb[..., 0],
        "stopx": stop_logits[..., 0], "stopz": gt_stop,
        "valid": valid,
        **{f"pid{k}": pfo_pid[..., k] for k in range(5)},
        **{f"poh{k}": poh[..., k] for k in range(5)},
    }
    sm = np.concatenate([pack_plane(planes[n]) for n in _PLANES], axis=1)

    l8 = np.zeros((P, P), np.float32)
    for j in range(J):
        blk = -PEN * np.tril(np.ones((T, T), np.float32)).T  # [k,t] = -96*(t>=k)
        l8[j * T : (j + 1) * T, j * T : (j + 1) * T] = blk
    l8 = l8.astype(NP_F8)
    ibf = np.eye(P, dtype=np.float32).astype(NP_BF16)

    # one-hot E (mask count) and D (selection) per core, fp8
    cj = cmin.reshape(N_CORES, J, HQ)
    pj = hit_to_pfo.reshape(N_CORES, J, HQ)
    wj = w.reshape(N_CORES, J, HQ)
    in_maps = []
    for c in range(N_CORES):
        E = np.zeros((P, HQ), NP_F8)
        D = np.zeros((P, HQ), NP_F8)
        for j in range(J):
            cc = cj[c, j]
            me = cc < T
            fs = np.nonzero(me)[0]
            E[j * T + cc[fs], fs] = 1.0
            fs = np.nonzero(wj[c, j])[0]
            D[j * T + pj[c, j][fs], fs] = 1.0
        xs = al[:, c * H : (c + 1) * H].reshape(T, J, HQ)
        xp = np.ascontiguousarray(xs.transpose(1, 0, 2).reshape(P, HQ))
        xhp = xp.astype(NP_BF16)
        in_maps.append(
            {"xh": xhp, "ed8": np.concatenate([E, D], axis=1), "l8": l8,
             "ibf": ibf, "sm": sm}
        )

    nc = _get_nc()
    res = run_bass_kernel_spmd(nc, in_maps, core_ids=list(range(N_CORES)))
    global last_result
    last_result = res

    # ---- host combine (float64) ----
    A_sum = 0.0
    B_sum = 0.0
    for c in range(N_CORES):
        pr = res.results[c]["partials"].astype(np.float64)
        A_sum += pr[:, 0:16].sum()
        B_sum += pr[:, 16:32].sum()
    loss_assign = (A_sum - B_sum) / assign_den

    pr0 = res.results[0]["partials"].astype(np.float64)
    loss_dir = pr0[:, 32].sum() / vcnt
    loss_mag = pr0[:, 33].sum() / vcnt
    loss_chg = pr0[:, 34].sum() / vcnt
    loss_pid = pr0[:, 35].sum() / vcnt
    loss_stop = pr0[:, 36].sum() / (T * B)

    total = (L_DIR * loss_dir + L_MAG * loss_mag + L_PID * loss_pid
             + L_CHG * loss_chg + L_ASN * loss_assign + L_STP * loss_stop)
    f = np.float32
    return (f(total), f(loss_dir), f(loss_mag), f(loss_pid), f(loss_chg),
            f(loss_assign), f(loss_stop))



# revision 2
# speedup vs baseline: 1.7396x; 1.7396x over previous
"""Trainium2 Bass kernel for nn_GATrAutoRegressorLoss.

Strategy (data-parallel over valid assignment elements, 8 cores):
  - The dominant cost is the assignment BCE over (T=32, N=500000) logits,
    but only ~half the (t, hit) pairs are valid (t < c(hit)).  The host
    packs exactly the valid elements (hit-major) into one flat stream,
    splits it evenly across the 8 cores as (128, NV=8064) fp8 tiles
    (padded with -100, which contributes exactly 0), so the device never
    touches masked elements and needs no masking machinery at all.
  - Per chunk the device computes u = exp(x) on the Scalar engine (ACT,
    1 elem/cycle/partition regardless of dtype), then folds softplus
    sums ln(1+u) through a pairwise product tree on the Vector engine:
    p2 = (1+u_L)(1+u_R) via two scalar_tensor_tensor ops, p4, p8 via
    two more multiplies (bf16 => 2x DVE rate), so the second ACT pass
    ln(p8) touches only 1/8 of the columns.  accum_out gives free
    per-partition row sums.  ACT busy/core ~= 1.125*NV cycles.
  - The "- x*z" BCE term is a single gather of one logit per hit; it and
    all denominators are exact host-side numpy (like the rest of the
    index bookkeeping: bincount, cumcount, scatter, argmax one-hots).
  - The small (T,B) losses (dir/mag/pid/charge/stop) are computed
    on-device from host-scattered bf16 planes, emitted between chunk 0
    and chunk 1 so they fill engine gaps during the DMA ramp.
  - Exp and Ln are pinned to the one ACT function table that contains
    both (see _Bacc) so the Scalar engine loads its table exactly once.
  - Per-core partial sums are returned and combined on the host in
    float64.  Any valid elements beyond device capacity (or |x| > 8
    outliers) are folded in exactly on the host; for the reference
    input distribution both sets are empty.
"""

import numpy as np

import concourse.bacc as bacc
import concourse.mybir as mybir
from concourse.tile import TileContext
from concourse.bass_utils import run_bass_kernel_spmd

F32 = mybir.dt.float32
BF16 = mybir.dt.bfloat16
F8 = mybir.dt.float8e4
NP_F8 = mybir.dt.np(F8)
NP_BF16 = mybir.dt.np(BF16)

T, B, N, NPFO = 32, 256, 500000, 4096
L_DIR, L_MAG, L_PID, L_CHG, L_ASN, L_STP = 1.0, 1.0, 1.0, 0.5, 1.0, 0.5

N_CORES = 8
P = 128                   # partitions
NV = 8064                 # packed valid columns per core
CAP = N_CORES * P * NV    # device element capacity
PAD = -100.0              # exp -> 0 exactly, ln(1+0) = 0
_CHUNKS = [(0, 1024), (1024, 2048), (3072, 2496), (5568, 2496)]
assert sum(w for _, w in _CHUNKS) == NV and all(w % 8 == 0 for _, w in _CHUNKS)
WMAX = max(w for _, w in _CHUNKS)

# small-loss planes, each (T*B,) flattened to (128, 64)
_PLANES = [
    "pm0", "pm1", "pm2", "gm0", "gm1", "gm2", "pp", "gp", "pch", "gch",
    "stopx", "stopz", "valid",
    "pid0", "pid1", "pid2", "pid3", "pid4",
    "poh0", "poh1", "poh2", "poh3", "poh4",
]
NPL = len(_PLANES)
SW = 64  # small-plane free width (T*B = 8192 = 128*64)

_nc_cache = None
last_result = None


class _Bacc(bacc.Bacc):
    """Bacc whose ACT-table chooser binds Exp/Ln to the one json table that
    contains both (natural_log_exp_and_others), so the Scalar engine never
    reloads function tables between exp and ln passes."""

    def insert_act_table_loads(self):
        from concourse.hw_specs import get_activation_tables

        has_activation = any(
            isinstance(i, mybir.InstActivation)
            for b in self.main_func.blocks
            for i in b.instructions
        )
        if not has_activation:
            return
        AF = mybir.ActivationFunctionType
        tables = []
        for name, fns in get_activation_tables(self.m.arch).items():
            if name != "natural_log_exp_and_others":
                fns = set(fns) - {AF.Exp, AF.Ln}
            tables.append((name, set(fns)))
        import bass_rust as _bass_rust

        _bass_rust.insert_act_table_loads(self, tables)


def _emit_small_losses(nc, sml, smt, accS):
    """(T,B) losses from bf16 planes; sums land in accS[:, 0:5]."""
    AF = mybir.ActivationFunctionType
    OP = mybir.AluOpType
    PLI = {n: i for i, n in enumerate(_PLANES)}

    def reg(name, k=1):
        i = PLI[name]
        return smt[:, i * SW : (i + k) * SW]

    _tmp_n = [0]

    def tmp(w=SW):
        _tmp_n[0] += 1
        nm = f"tmp{_tmp_n[0]}"
        return sml.tile([P, w], F32, name=nm, tag=nm)

    def red(ap, k, op=OP.add):
        o = tmp()
        nc.vector.tensor_reduce(
            out=o[:],
            in_=ap.rearrange("p (k j) -> p j k", k=k),
            axis=mybir.AxisListType.X,
            op=op,
        )
        return o

    valid = reg("valid")

    # --- direction loss: 1 - cos_sim, masked
    sqv = tmp(6 * SW)
    nc.scalar.activation(out=sqv[:], in_=reg("pm0", 6), func=AF.Square)
    ssb = tmp(2 * SW)
    nc.vector.tensor_reduce(
        out=ssb[:, 0:SW],
        in_=sqv[:, 0 : 3 * SW].rearrange("p (k j) -> p j k", k=3),
        axis=mybir.AxisListType.X, op=OP.add,
    )
    nc.vector.tensor_reduce(
        out=ssb[:, SW : 2 * SW],
        in_=sqv[:, 3 * SW : 6 * SW].rearrange("p (k j) -> p j k", k=3),
        axis=mybir.AxisListType.X, op=OP.add,
    )
    lnb = tmp(2 * SW)
    nc.scalar.activation(out=lnb[:], in_=ssb[:], func=AF.Ln)
    srb = tmp(2 * SW)
    nc.scalar.activation(out=srb[:], in_=lnb[:], func=AF.Exp, scale=0.5)
    nc.vector.tensor_scalar(
        out=srb[:], in0=srb[:], scalar1=1e-8, scalar2=None, op0=OP.max,
    )
    nc.vector.reciprocal(out=srb[:], in_=srb[:])
    dmul = tmp(3 * SW)
    nc.vector.tensor_mul(dmul[:], reg("pm0", 3), reg("gm0", 3))
    dot = red(dmul[:], 3)
    nc.vector.tensor_mul(dot[:], dot[:], srb[:, 0:SW])
    nc.vector.tensor_mul(dot[:], dot[:], srb[:, SW : 2 * SW])
    cv = tmp()
    nc.vector.tensor_mul(cv[:], dot[:], valid)
    o1 = tmp()
    nc.vector.scalar_tensor_tensor(
        out=o1[:], in0=cv[:], scalar=-1.0, in1=valid,
        op0=OP.mult, op1=OP.add, accum_out=accS[:, 0:1],
    )

    # --- magnitude / charge (masked squared diffs)
    dif = tmp(2 * SW)
    nc.vector.tensor_sub(dif[:, 0:SW], reg("pp"), reg("gp"))
    nc.vector.tensor_sub(dif[:, SW : 2 * SW], reg("pch"), reg("gch"))
    dsq = tmp(2 * SW)
    nc.scalar.activation(out=dsq[:], in_=dif[:], func=AF.Square)
    for col, sl in ((1, slice(0, SW)), (2, slice(SW, 2 * SW))):
        o = tmp()
        nc.vector.scalar_tensor_tensor(
            out=o[:], in0=dsq[:, sl], scalar=1.0, in1=valid,
            op0=OP.mult, op1=OP.mult, accum_out=accS[:, col : col + 1],
        )

    # --- pid cross entropy (direct logsumexp; |logits| small)
    pexp = tmp(5 * SW)
    nc.scalar.activation(out=pexp[:], in_=reg("pid0", 5), func=AF.Exp)
    se = red(pexp[:], 5)
    lse = tmp()
    nc.scalar.activation(out=lse[:], in_=se[:], func=AF.Ln)
    xm = tmp(5 * SW)
    nc.vector.tensor_mul(xm[:], reg("pid0", 5), reg("poh0", 5))
    xcls = red(xm[:], 5)
    u1 = tmp()
    nc.vector.scalar_tensor_tensor(
        out=u1[:], in0=xcls[:], scalar=-1.0, in1=lse[:],
        op0=OP.mult, op1=OP.add,
    )
    o2 = tmp()
    nc.vector.scalar_tensor_tensor(
        out=o2[:], in0=u1[:], scalar=1.0, in1=valid,
        op0=OP.mult, op1=OP.mult, accum_out=accS[:, 3:4],
    )

    # --- stop BCE over all (T,B)
    usp = tmp()
    nc.scalar.activation(out=usp[:], in_=reg("stopx"), func=AF.Exp)
    spv = tmp()
    nc.scalar.activation(out=spv[:], in_=usp[:], func=AF.Ln, bias=1.0)
    xz = tmp()
    nc.vector.tensor_mul(xz[:], reg("stopx"), reg("stopz"))
    o3 = tmp()
    nc.vector.scalar_tensor_tensor(
        out=o3[:], in0=xz[:], scalar=-1.0, in1=spv[:],
        op0=OP.mult, op1=OP.add, accum_out=accS[:, 4:5],
    )


def _gen():
    nc = _Bacc(None, target_bir_lowering=False, debug=True)
    xq = nc.dram_tensor("xq", [P, NV], F8, kind="ExternalInput")
    sm = nc.dram_tensor("sm", [P, NPL * SW], BF16, kind="ExternalInput")
    partials = nc.dram_tensor("partials", [P, 16], F32, kind="ExternalOutput")

    AF = mybir.ActivationFunctionType
    OP = mybir.AluOpType

    with TileContext(nc) as tc:
        with (
            tc.tile_pool(name="cst", bufs=1) as cst,
            tc.tile_pool(name="io", bufs=4) as io,
            tc.tile_pool(name="wk", bufs=2) as wk,
            tc.tile_pool(name="sml", bufs=1) as sml,
        ):
            acc = cst.tile([P, 16], F32)
            nc.vector.memset(acc[:], 0.0)
            accA = acc[:, 0:4]    # per-chunk softplus sums
            accS = acc[:, 4:9]    # dir, mag, chg, pid, stop

            # stage all input DMAs up front (x chunk 0 first, so ACT
            # starts early; small planes ride behind it)
            xts = []
            smt = sml.tile([P, NPL * SW], BF16)
            for ci, (c0, w) in enumerate(_CHUNKS):
                xt = io.tile([P, WMAX], F8, tag="x8")
                nc.sync.dma_start(out=xt[:, :w], in_=xq[:, c0 : c0 + w])
                xts.append(xt)
                if ci == 0:
                    nc.sync.dma_start(out=smt[:], in_=sm[:])

            for ci, (c0, w) in enumerate(_CHUNKS):
                w2, w4, w8 = w // 2, w // 4, w // 8
                xt = xts[ci]
                ut = wk.tile([P, WMAX], BF16, tag="ut")
                nc.scalar.activation(out=ut[:, :w], in_=xt[:, :w], func=AF.Exp)
                # p2 = (1+u_L)(1+u_R) in two DVE ops
                rt = wk.tile([P, WMAX // 2], BF16, tag="rt")
                nc.vector.scalar_tensor_tensor(
                    out=rt[:, :w2], in0=ut[:, :w2], scalar=1.0,
                    in1=ut[:, w2:w], op0=OP.add, op1=OP.mult,
                )
                p2 = wk.tile([P, WMAX // 2], BF16, tag="p2")
                nc.vector.scalar_tensor_tensor(
                    out=p2[:, :w2], in0=ut[:, :w2], scalar=1.0,
                    in1=rt[:, :w2], op0=OP.add, op1=OP.add,
                )
                p4 = wk.tile([P, WMAX // 4], BF16, tag="p4")
                nc.vector.tensor_mul(p4[:, :w4], p2[:, :w4], p2[:, w4:w2])
                p8 = wk.tile([P, WMAX // 8], BF16, tag="p8")
                nc.vector.tensor_mul(p8[:, :w8], p4[:, :w8], p4[:, w8:w4])
                s8 = wk.tile([P, WMAX // 8], BF16, tag="s8")
                nc.scalar.activation(
                    out=s8[:, :w8], in_=p8[:, :w8], func=AF.Ln,
                    accum_out=accA[:, ci : ci + 1],
                )
                if ci == 0:
                    _emit_small_losses(nc, sml, smt, accS)

            nc.sync.dma_start(out=partials[:], in_=acc[:])
    nc.finalize()
    return nc


def _get_nc():
    global _nc_cache
    if _nc_cache is None:
        _nc_cache = _gen()
    return _nc_cache


def _cumcount(gb):
    n = gb.shape[0]
    order = np.argsort(gb, kind="stable")
    sb = gb[order]
    first = np.searchsorted(sb, sb, side="left")
    cum = np.arange(n) - first
    out = np.zeros(n, dtype=np.int64)
    out[order] = cum
    return out


def kernel(**inputs):
    pfo_momentum = np.asarray(inputs["pfo_momentum"], np.float32)
    pfo_p_mod = np.asarray(inputs["pfo_p_mod"], np.float32)
    pfo_pid = np.asarray(inputs["pfo_pid"], np.float32)
    pfo_charge = np.asarray(inputs["pfo_charge"], np.float32)
    al = np.asarray(inputs["assignments_logits"], np.float32).reshape(T, N)
    stop_logits = np.asarray(inputs["stop_logits"], np.float32)
    gt_momentum = np.asarray(inputs["gt_momentum"], np.float32)
    gt_p_mod = np.asarray(inputs["gt_p_mod"], np.float32)
    gt_pid = np.asarray(inputs["gt_pid"], np.float32)
    gt_charge = np.asarray(inputs["gt_charge"], np.float32)
    gt_batch = np.asarray(inputs["gt_batch"]).astype(np.int64)
    hit_to_pfo = np.asarray(inputs["hit_to_pfo"]).astype(np.int64)
    hit_batch = np.asarray(inputs["hit_batch"]).astype(np.int64)

    # ---- host index bookkeeping ----
    ppe = np.bincount(gt_batch, minlength=B)[:B]                  # (B,)
    cmin = np.minimum(ppe[hit_batch], T).astype(np.int64)         # (N,)
    w = hit_to_pfo < cmin                                         # (N,) bool
    assign_den = max(float(cmin.sum()), 1.0)

    # exact selection term: sum over valid hits of x[hit_to_pfo[h], h]
    sel_sum = float(al[hit_to_pfo, np.arange(N)][w].sum(dtype=np.float64))

    # ---- pack valid assignment logits (hit-major) ----
    alT = np.ascontiguousarray(al.T)                              # (N, T)
    maskT = np.arange(T, dtype=np.int64)[None, :] < cmin[:, None]
    flat = alT[maskT]                                             # (V,) f32
    spill = 0.0
    big = np.abs(flat) > 8.0
    if big.any():
        bv = flat[big].astype(np.float64)
        spill += float(np.logaddexp(0.0, bv).sum())
        flat = np.where(big, np.float32(PAD), flat)
    if flat.shape[0] > CAP:
        rest = flat[CAP:].astype(np.float64)
        keep = rest > PAD + 1.0  # skip already-padded outliers
        spill += float(np.logaddexp(0.0, rest[keep]).sum())
        flat = flat[:CAP]
    arr = np.full(CAP, PAD, np.float32)
    arr[: flat.shape[0]] = flat
    x8 = arr.astype(NP_F8).reshape(N_CORES, NV, P)

    step_idx = _cumcount(gt_batch)
    keep = step_idx < T
    si, gb = step_idx[keep], gt_batch[keep]

    def scat(vals):
        out = np.zeros((T, B) + vals.shape[1:], np.float32)
        out[si, gb] = vals[keep]
        return out

    gt_mom_tb = scat(gt_momentum)
    gt_pmod_tb = scat(gt_p_mod)
    gt_pid_tb = scat(gt_pid)
    gt_chg_tb = scat(gt_charge)

    steps = np.arange(T)[:, None]
    valid = (steps < ppe[None, :]).astype(np.float32)             # (T,B)
    vcnt = max(float(valid.sum()), 1.0)
    gt_stop = (steps >= ppe[None, :]).astype(np.float32)
    gt_cls = np.argmax(gt_pid_tb, axis=-1)                        # (T,B)
    poh = np.zeros((T, B, 5), np.float32)
    np.put_along_axis(poh, gt_cls[..., None], 1.0, axis=-1)

    def pack_plane(a):
        return np.ascontiguousarray(a.reshape(P, SW))

    planes = {
        "pm0": pfo_momentum[..., 0], "pm1": pfo_momentum[..., 1],
        "pm2": pfo_momentum[..., 2],
        "gm0": gt_mom_tb[..., 0], "gm1": gt_mom_tb[..., 1],
        "gm2": gt_mom_tb[..., 2],
        "pp": pfo_p_mod[..., 0], "gp": gt_pmod_tb[..., 0],
        "pch": pfo_charge[..., 0], "gch": gt_chg_tb[..., 0],
        "stopx": stop_logits[..., 0], "stopz": gt_stop,
        "valid": valid,
        **{f"pid{k}": pfo_pid[..., k] for k in range(5)},
        **{f"poh{k}": poh[..., k] for k in range(5)},
    }
    sm = np.concatenate(
        [pack_plane(planes[n]) for n in _PLANES], axis=1
    ).astype(NP_BF16)

    in_maps = [
        {"xq": np.ascontiguousarray(x8[c].T), "sm": sm}
        for c in range(N_CORES)
    ]

    nc = _get_nc()
    res = run_bass_kernel_spmd(nc, in_maps, core_ids=list(range(N_CORES)))
    global last_result
    last_result = res

    # ---- host combine (float64) ----
    A_sum = 0.0
    for c in range(N_CORES):
        pr = res.results[c]["partials"].astype(np.float64)
        A_sum += pr[:, 0:4].sum()
    loss_assign = (A_sum + spill - sel_sum) / assign_den

    pr0 = res.results[0]["partials"].astype(np.float64)
    loss_dir = pr0[:, 4].sum() / vcnt
    loss_mag = pr0[:, 5].sum() / vcnt
    loss_chg = pr0[:, 6].sum() / vcnt
    loss_pid = pr0[:, 7].sum() / vcnt
    loss_stop = pr0[:, 8].sum() / (T * B)

    total = (L_DIR * loss_dir + L_MAG * loss_mag + L_PID * loss_pid
             + L_CHG * loss_chg + L_ASN * loss_assign + L_STP * loss_stop)
    f = np.float32
    return (f(total), f(loss_dir), f(loss_mag), f(loss_pid), f(loss_chg),
            f(loss_assign), f(loss_stop))


# revision 4
# speedup vs baseline: 2.0330x; 1.1686x over previous
"""Trainium2 Bass kernel for nn_GATrAutoRegressorLoss.

Strategy (data-parallel over valid assignment elements, 8 cores):
  - The dominant cost is the assignment BCE over (T=32, N=500000) logits,
    but only ~half the (t, hit) pairs are valid (t < c(hit)).  The host
    packs exactly the valid elements (hit-major) into one flat stream,
    splits it evenly across the 8 cores as (128, NV=8064) fp8 tiles
    (padded with -100, which contributes exactly 0), so the device never
    touches masked elements and needs no masking machinery at all.
  - Per chunk the device computes u = exp(x) on the Scalar engine (ACT,
    1 elem/cycle/partition regardless of dtype), then folds the softplus
    sum  sum ln(1+u)  through a pairwise product tree on the Vector
    engine: v = u+1 via tensor_scalar (4x DVE mode for packed bf16),
    then v_L*v_R tensor_tensor multiplies (2x mode) down to 1/16 of the
    columns, so the second ACT pass ln(p16) is a single small
    instruction with a free accum_out row-sum.  scalar_tensor_tensor is
    avoided in the hot path (it has no DVE perf modes = half rate).
  - The "- x*z" BCE term is a single gather of one logit per hit; it and
    all denominators are exact host-side numpy (like the rest of the
    index bookkeeping: bincount, cumcount, scatter, argmax one-hots,
    direction dot/inverse-norm planes, pid class gather).
  - The small (T,B) losses (dir/mag/pid/charge/stop) are computed
    on-device from host-prepared bf16 planes, emitted between chunks so
    they fill engine gaps during the DMA ramp.
  - Exp and Ln are pinned to the one ACT function table that contains
    both (see _Bacc) so the Scalar engine loads its table exactly once.
  - Per-core partial sums are returned and combined on the host in
    float64.  Any valid elements beyond device capacity (or |x| > 4.5
    outliers, which also bounds the product tree far below overflow) are
    folded in exactly on the host; for the reference input distribution
    the overflow set is ~10 elements and the tree maxes out around 1e7.
"""

import numpy as np

import concourse.bacc as bacc
import concourse.mybir as mybir
from concourse.tile import TileContext
from concourse.bass_utils import run_bass_kernel_spmd

F32 = mybir.dt.float32
BF16 = mybir.dt.bfloat16
F8 = mybir.dt.float8e4
NP_F8 = mybir.dt.np(F8)
NP_BF16 = mybir.dt.np(BF16)

T, B, N, NPFO = 32, 256, 500000, 4096
L_DIR, L_MAG, L_PID, L_CHG, L_ASN, L_STP = 1.0, 1.0, 1.0, 0.5, 1.0, 0.5

N_CORES = 8
P = 128                   # partitions
NV = 8064                 # packed valid columns per core
CAP = N_CORES * P * NV    # device element capacity
PAD = -100.0              # exp -> 0 exactly, ln(1+0) = 0
XCLIP = 4.5               # |x| above this handled on host (tree stays tiny)
_CHUNKS = [(0, 256), (256, 2496), (2752, 2496), (5248, 2496), (7744, 320)]
assert sum(w for _, w in _CHUNKS) == NV and all(w % 16 == 0 for _, w in _CHUNKS)
WMAX = max(w for _, w in _CHUNKS)
N16 = NV // 16            # final ln width

# small-loss planes, each (T*B,) flattened to (128, 64)
_PLANES = [
    "dot", "rnn", "pp", "gp", "pch", "gch", "stopx", "stopz", "valid",
    "pid0", "pid1", "pid2", "pid3", "pid4",
]
NPL = len(_PLANES)
SW = 64  # small-plane free width (T*B = 8192 = 128*64)

_nc_cache = None
last_result = None


class _Bacc(bacc.Bacc):
    """Bacc whose ACT-table chooser binds Exp/Ln to the one json table that
    contains both (natural_log_exp_and_others), so the Scalar engine never
    reloads function tables between exp and ln passes."""

    def insert_act_table_loads(self):
        from concourse.hw_specs import get_activation_tables

        has_activation = any(
            isinstance(i, mybir.InstActivation)
            for b in self.main_func.blocks
            for i in b.instructions
        )
        if not has_activation:
            return
        AF = mybir.ActivationFunctionType
        tables = []
        for name, fns in get_activation_tables(self.m.arch).items():
            if name != "natural_log_exp_and_others":
                fns = set(fns) - {AF.Exp, AF.Ln}
            tables.append((name, set(fns)))
        import bass_rust as _bass_rust

        _bass_rust.insert_act_table_loads(self, tables)


def _emit_small_losses(nc, sml, smt, accS):
    """(T,B) losses from bf16 planes; sums land in accS[:, 0:5].

    accS[0] = sum(-cos*valid)         (host adds vcnt and divides)
    accS[1] = sum((pp-gp)^2 * valid)
    accS[2] = sum((pch-gch)^2 * valid)
    accS[3] = sum(lse * valid)        (host subtracts the class gather)
    accS[4] = sum(softplus(stopx) - stopx*stopz)
    """
    AF = mybir.ActivationFunctionType
    OP = mybir.AluOpType
    PLI = {n: i for i, n in enumerate(_PLANES)}

    def reg(name, k=1):
        i = PLI[name]
        return smt[:, i * SW : (i + k) * SW]

    _tmp_n = [0]

    def tmp(w=SW, dt=F32):
        _tmp_n[0] += 1
        nm = f"tmp{_tmp_n[0]}"
        return sml.tile([P, w], dt, name=nm, tag=nm)

    valid = reg("valid")

    # --- pid partial: sum(valid * logsumexp(pid)) --------------------
    pexp = tmp(5 * SW)
    nc.scalar.activation(out=pexp[:], in_=reg("pid0", 5), func=AF.Exp)
    se = tmp()
    nc.vector.tensor_reduce(
        out=se[:], in_=pexp[:].rearrange("p (k j) -> p j k", k=5),
        axis=mybir.AxisListType.X, op=OP.add,
    )
    lse = tmp()
    nc.scalar.activation(out=lse[:], in_=se[:], func=AF.Ln)
    o2 = tmp()
    nc.vector.scalar_tensor_tensor(
        out=o2[:], in0=lse[:], scalar=1.0, in1=valid,
        op0=OP.mult, op1=OP.mult, accum_out=accS[:, 3:4],
    )

    # --- stop BCE over all (T,B) ------------------------------------
    usp = tmp()
    nc.scalar.activation(out=usp[:], in_=reg("stopx"), func=AF.Exp)
    spv = tmp()
    nc.scalar.activation(out=spv[:], in_=usp[:], func=AF.Ln, bias=1.0)
    xz = tmp(dt=BF16)
    nc.vector.tensor_mul(xz[:], reg("stopx"), reg("stopz"))
    o3 = tmp()
    nc.vector.scalar_tensor_tensor(
        out=o3[:], in0=xz[:], scalar=-1.0, in1=spv[:],
        op0=OP.mult, op1=OP.add, accum_out=accS[:, 4:5],
    )

    # --- direction: sum(-cos*valid) from host dot & 1/(|a||b|) ------
    cos = tmp(dt=BF16)
    nc.vector.tensor_mul(cos[:], reg("dot"), reg("rnn"))
    cv = tmp(dt=BF16)
    nc.vector.tensor_mul(cv[:], cos[:], valid)
    o1 = tmp()
    nc.vector.tensor_scalar(
        out=o1[:], in0=cv[:], scalar1=-1.0, scalar2=0.0, op0=OP.mult,
        op1=OP.add, accum_out=accS[:, 0:1],
    )

    # --- magnitude / charge (masked squared diffs) ------------------
    dif = tmp(2 * SW, dt=BF16)
    nc.vector.tensor_sub(dif[:, 0:SW], reg("pp"), reg("gp"))
    nc.vector.tensor_sub(dif[:, SW : 2 * SW], reg("pch"), reg("gch"))
    dsq = tmp(2 * SW, dt=BF16)
    nc.vector.tensor_mul(dsq[:], dif[:], dif[:])
    for col, sl in ((1, slice(0, SW)), (2, slice(SW, 2 * SW))):
        o = tmp()
        nc.vector.scalar_tensor_tensor(
            out=o[:], in0=dsq[:, sl], scalar=1.0, in1=valid,
            op0=OP.mult, op1=OP.mult, accum_out=accS[:, col : col + 1],
        )


def _gen():
    nc = _Bacc(None, target_bir_lowering=False, debug=True)
    xq = nc.dram_tensor("xq", [P, NV], F8, kind="ExternalInput")
    sm = nc.dram_tensor("sm", [P, NPL * SW], BF16, kind="ExternalInput")
    partials = nc.dram_tensor("partials", [P, 16], F32, kind="ExternalOutput")

    AF = mybir.ActivationFunctionType
    OP = mybir.AluOpType

    with TileContext(nc) as tc:
        with (
            tc.tile_pool(name="cst", bufs=1) as cst,
            tc.tile_pool(name="io", bufs=5) as io,
            tc.tile_pool(name="wk", bufs=2) as wk,
            tc.tile_pool(name="sml", bufs=1) as sml,
        ):
            acc = cst.tile([P, 16], F32)
            nc.vector.memset(acc[:], 0.0)
            accA = acc[:, 0:1]    # softplus sum (via final ln accum)
            accS = acc[:, 4:9]    # dir, mag, chg, pid, stop
            p16 = cst.tile([P, N16], BF16)

            # stage all input DMAs up front (tiny x chunk 0 first, so ACT
            # starts early; small planes ride behind it)
            xts = []
            smt = sml.tile([P, NPL * SW], BF16)
            for ci, (c0, w) in enumerate(_CHUNKS):
                xt = io.tile([P, WMAX], F8, tag="x8")
                nc.sync.dma_start(out=xt[:, :w], in_=xq[:, c0 : c0 + w])
                xts.append(xt)
                if ci == 0:
                    nc.sync.dma_start(out=smt[:], in_=sm[:])

            o16 = 0
            for ci, (c0, w) in enumerate(_CHUNKS):
                w2, w4, w8, w16 = w // 2, w // 4, w // 8, w // 16
                xt = xts[ci]
                ut = wk.tile([P, WMAX], BF16, tag="ut")
                nc.scalar.activation(out=ut[:, :w], in_=xt[:, :w], func=AF.Exp)
                vt = wk.tile([P, WMAX], BF16, tag="vt")
                nc.vector.tensor_scalar(
                    out=vt[:, :w], in0=ut[:, :w], scalar1=1.0, scalar2=None,
                    op0=OP.add,
                )
                p2 = wk.tile([P, WMAX // 2], BF16, tag="p2")
                nc.vector.tensor_mul(p2[:, :w2], vt[:, :w2], vt[:, w2:w])
                p4 = wk.tile([P, WMAX // 4], BF16, tag="p4")
                nc.vector.tensor_mul(p4[:, :w4], p2[:, :w4], p2[:, w4:w2])
                p8 = wk.tile([P, WMAX // 8], BF16, tag="p8")
                nc.vector.tensor_mul(p8[:, :w8], p4[:, :w8], p4[:, w8:w4])
                nc.vector.tensor_mul(
                    p16[:, o16 : o16 + w16], p8[:, :w16], p8[:, w16:w8]
                )
                o16 += w16
                if ci == 1:
                    _emit_small_losses(nc, sml, smt, accS)

            s16 = cst.tile([P, N16], BF16)
            nc.scalar.activation(
                out=s16[:], in_=p16[:], func=AF.Ln, accum_out=accA[:, 0:1],
            )

            nc.sync.dma_start(out=partials[:], in_=acc[:])
    nc.finalize()
    return nc


def _get_nc():
    global _nc_cache
    if _nc_cache is None:
        _nc_cache = _gen()
    return _nc_cache


def _cumcount(gb):
    n = gb.shape[0]
    order = np.argsort(gb, kind="stable")
    sb = gb[order]
    first = np.searchsorted(sb, sb, side="left")
    cum = np.arange(n) - first
    out = np.zeros(n, dtype=np.int64)
    out[order] = cum
    return out


def kernel(**inputs):
    pfo_momentum = np.asarray(inputs["pfo_momentum"], np.float32)
    pfo_p_mod = np.asarray(inputs["pfo_p_mod"], np.float32)
    pfo_pid = np.asarray(inputs["pfo_pid"], np.float32)
    pfo_charge = np.asarray(inputs["pfo_charge"], np.float32)
    al = np.asarray(inputs["assignments_logits"], np.float32).reshape(T, N)
    stop_logits = np.asarray(inputs["stop_logits"], np.float32)
    gt_momentum = np.asarray(inputs["gt_momentum"], np.float32)
    gt_p_mod = np.asarray(inputs["gt_p_mod"], np.float32)
    gt_pid = np.asarray(inputs["gt_pid"], np.float32)
    gt_charge = np.asarray(inputs["gt_charge"], np.float32)
    gt_batch = np.asarray(inputs["gt_batch"]).astype(np.int64)
    hit_to_pfo = np.asarray(inputs["hit_to_pfo"]).astype(np.int64)
    hit_batch = np.asarray(inputs["hit_batch"]).astype(np.int64)

    # ---- host index bookkeeping ----
    ppe = np.bincount(gt_batch, minlength=B)[:B]                  # (B,)
    cmin = np.minimum(ppe[hit_batch], T).astype(np.int64)         # (N,)
    w = hit_to_pfo < cmin                                         # (N,) bool
    assign_den = max(float(cmin.sum()), 1.0)

    # exact selection term: sum over valid hits of x[hit_to_pfo[h], h]
    sel_sum = float(al[hit_to_pfo, np.arange(N)][w].sum(dtype=np.float64))

    # ---- pack valid assignment logits (hit-major) ----
    alT = np.ascontiguousarray(al.T)                              # (N, T)
    maskT = np.arange(T, dtype=np.int64)[None, :] < cmin[:, None]
    flat = alT[maskT]                                             # (V,) f32
    spill = 0.0
    big = np.abs(flat) > XCLIP
    if big.any():
        bv = flat[big].astype(np.float64)
        spill += float(np.logaddexp(0.0, bv).sum())
        flat = np.where(big, np.float32(PAD), flat)
    if flat.shape[0] > CAP:
        rest = flat[CAP:].astype(np.float64)
        keep = rest > PAD + 1.0  # skip already-padded outliers
        spill += float(np.logaddexp(0.0, rest[keep]).sum())
        flat = flat[:CAP]
    arr = np.full(CAP, PAD, np.float32)
    arr[: flat.shape[0]] = flat
    x8 = arr.astype(NP_F8).reshape(N_CORES, NV, P)

    step_idx = _cumcount(gt_batch)
    keep = step_idx < T
    si, gb = step_idx[keep], gt_batch[keep]

    def scat(vals):
        out = np.zeros((T, B) + vals.shape[1:], np.float32)
        out[si, gb] = vals[keep]
        return out

    gt_mom_tb = scat(gt_momentum)
    gt_pmod_tb = scat(gt_p_mod)
    gt_pid_tb = scat(gt_pid)
    gt_chg_tb = scat(gt_charge)

    steps = np.arange(T)[:, None]
    valid = (steps < ppe[None, :]).astype(np.float32)             # (T,B)
    vcnt = max(float(valid.sum()), 1.0)
    gt_stop = (steps >= ppe[None, :]).astype(np.float32)
    gt_cls = np.argmax(gt_pid_tb, axis=-1)                        # (T,B)
    # exact pid class-logit gather (host part of the cross entropy)
    pid_sel = float(
        (np.take_along_axis(pfo_pid, gt_cls[..., None], axis=-1)[..., 0]
         * valid).sum(dtype=np.float64)
    )
    # direction dot & inverse-norm product planes
    dot = (pfo_momentum * gt_mom_tb).sum(axis=-1)                 # (T,B)
    na = np.maximum(np.linalg.norm(pfo_momentum, axis=-1), 1e-8)
    nb = np.maximum(np.linalg.norm(gt_mom_tb, axis=-1), 1e-8)
    rnn = (1.0 / (na * nb)).astype(np.float32)

    def pack_plane(a):
        return np.ascontiguousarray(a.reshape(P, SW))

    planes = {
        "dot": dot, "rnn": rnn,
        "pp": pfo_p_mod[..., 0], "gp": gt_pmod_tb[..., 0],
        "pch": pfo_charge[..., 0], "gch": gt_chg_tb[..., 0],
        "stopx": stop_logits[..., 0], "stopz": gt_stop,
        "valid": valid,
        **{f"pid{k}": pfo_pid[..., k] for k in range(5)},
    }
    sm = np.concatenate(
        [pack_plane(planes[n]) for n in _PLANES], axis=1
    ).astype(NP_BF16)

    in_maps = [
        {"xq": np.ascontiguousarray(x8[c].T), "sm": sm}
        for c in range(N_CORES)
    ]

    nc = _get_nc()
    res = run_bass_kernel_spmd(nc, in_maps, core_ids=list(range(N_CORES)))
    global last_result
    last_result = res

    # ---- host combine (float64) ----
    A_sum = 0.0
    for c in range(N_CORES):
        pr = res.results[c]["partials"].astype(np.float64)
        A_sum += pr[:, 0].sum()
    loss_assign = (A_sum + spill - sel_sum) / assign_den

    pr0 = res.results[0]["partials"].astype(np.float64)
    loss_dir = (vcnt + pr0[:, 4].sum()) / vcnt
    loss_mag = pr0[:, 5].sum() / vcnt
    loss_chg = pr0[:, 6].sum() / vcnt
    loss_pid = (pr0[:, 7].sum() - pid_sel) / vcnt
    loss_stop = pr0[:, 8].sum() / (T * B)

    total = (L_DIR * loss_dir + L_MAG * loss_mag + L_PID * loss_pid
             + L_CHG * loss_chg + L_ASN * loss_assign + L_STP * loss_stop)
    f = np.float32
    return (f(total), f(loss_dir), f(loss_mag), f(loss_pid), f(loss_chg),
            f(loss_assign), f(loss_stop))
